# revision 1
# baseline (speedup 1.0000x reference)
"""Trainium2 Bass kernel for nn_AaD_MAPU (retrieval kNN + KL attraction / dispersion loss).

Reference computation:
    softmax_out = softmax(predictions)                      [B,C]
    f_norm      = l2_normalize(features)                    [B,D]
    fb          = fea_bank with rows trg_idx <- f_norm      [N,D]
    sb          = score_bank with rows trg_idx <- softmax   [N,C]
    distance    = f_norm @ fb.T                             [B,N]
    idx         = top_k(distance, K+1); idx_near = idx[:,1:]
    score_near  = sb[idx_near]                              [B,K,C]
    loss        = sum(score_near * (log(score_near) - softmax[:,None,:])) / B
    neg_pred    = mean(rowsum(softmax @ softmax.T - diag))
    out         = loss + neg_pred

Device strategy (8 NeuronCores, bank rows sharded):
  - Pad bank to 100352 rows; each core owns 12544 contiguous rows.
  - Stream the fp32 bank shard with a casting SWDGE DMA (fp32 -> bf16).
  - Transpose each [128j, 128d] block on the TensorEngine by multiplying
    against an identity (out = tile.T @ I), giving fbT [d, j] tiles.
  - bf16 matmul vs pre-transposed f_norm.T -> PSUM fp32 [128b, 512j].
  - VectorE segmented reduce_max (8-wide segments) straight from PSUM
    -> per-row segment maxima [128, 1568] fp32 per batch chunk.
  - VectorE max8 + find_index8 -> per-row top-8 (segment max, segment id).
Host merges 8 cores x 8 candidates per row, resolves the argmax position
inside each 8-wide winning segment with a handful of fp32 dots, drops the
top-1 (reference drops idx[:,0]), gathers scores and reduces the loss.
"""

import os
from contextlib import ExitStack

import numpy as np

import concourse.bass as bass
import concourse.tile as tile
from concourse import bacc, mybir
from concourse.bass_utils import run_bass_kernel_spmd
from concourse.masks import make_identity

# Problem constants (hardcoded per contest rules).
B, D, N, C, K = 512, 512, 100000, 64, 5
EPS = 1e-12
NCORES = 8
NSHARD = 12544            # padded bank rows per core (98 * 128)
NPAD = NSHARD * NCORES    # 100352
SEG = 8                   # segment width for the two-level top-k
NSEG = NSHARD // SEG      # 1568 segments per core per row
BCH = 4                   # batch chunks of 128 rows
JT = 512                  # j-tile width
NJT = 25                  # 24 full tiles + one 256-wide tile

_F32 = mybir.dt.float32
_BF16 = mybir.dt.bfloat16
_U32 = mybir.dt.uint32

_cache = {}


def _build_module():
    nc = bacc.Bacc("TRN2", target_bir_lowering=False, debug=False,
                   num_devices=NCORES)
    fb = nc.dram_tensor("fb", [NSHARD, D], _F32, kind="ExternalInput").ap()
    fnt = nc.dram_tensor("fnt", [128, 4 * B], _F32, kind="ExternalInput").ap()
    val_out = nc.dram_tensor("val_out", [BCH, 128, 8], _F32,
                             kind="ExternalOutput").ap()
    idx_out = nc.dram_tensor("idx_out", [BCH, 128, 8], _U32,
                             kind="ExternalOutput").ap()

    with tile.TileContext(nc) as tc, ExitStack() as ctx:
        const = ctx.enter_context(tc.tile_pool(name="const", bufs=1))
        fbn_pool = ctx.enter_context(tc.tile_pool(name="fbn", bufs=3))
        fbt_pool = ctx.enter_context(tc.tile_pool(name="fbt", bufs=3))
        tp_pool = ctx.enter_context(tc.tile_pool(name="tp", bufs=2, space="PSUM"))
        dp_pool = ctx.enter_context(tc.tile_pool(name="dp", bufs=3, space="PSUM"))
        out_pool = ctx.enter_context(tc.tile_pool(name="outs", bufs=2))

        # f_norm.T packed on host as [dp, dc*B + b] -> view [128, 4, B]
        fnt_sb = const.tile([128, 4, B], _BF16)
        nc.gpsimd.dma_start(fnt_sb[:], fnt.rearrange("p (c b) -> p c b", c=4))

        ident = const.tile([128, 128], _BF16)
        make_identity(nc, ident[:])

        segmax = const.tile([128, BCH, NSEG], _F32)

        for t in range(NJT):
            j0 = t * JT
            W = min(JT, NSHARD - j0)
            S = W // 128

            # bank tile, natural layout [j, d], cast fp32 -> bf16 in the DMA
            fbn = fbn_pool.tile([128, 4, D], _BF16, tag="fbn")
            nc.gpsimd.dma_start(
                fbn[:, :S],
                fb[j0:j0 + W].rearrange("(s p) d -> p s d", p=128),
            )

            # transpose to [d, j] via PE: out = block.T @ I
            fbt = fbt_pool.tile([128, 4, JT], _BF16, tag="fbt")
            for dc in range(4):
                pt = tp_pool.tile([128, JT], _F32, tag="pt")
                for s in range(S):
                    nc.tensor.matmul(
                        pt[:, s * 128:(s + 1) * 128],
                        lhsT=fbn[:, s, dc * 128:(dc + 1) * 128],
                        rhs=ident[:],
                        start=True, stop=True,
                    )
                nc.scalar.copy(out=fbt[:, dc, :W], in_=pt[:, :W])

            # distance tiles + fused segmented max
            for bc in range(BCH):
                dp = dp_pool.tile([128, JT], _F32, tag="dp")
                for dc in range(4):
                    nc.tensor.matmul(
                        dp[:, :W],
                        lhsT=fnt_sb[:, dc, bc * 128:(bc + 1) * 128],
                        rhs=fbt[:, dc, :W],
                        start=(dc == 0), stop=(dc == 3),
                    )
                nc.vector.tensor_reduce(
                    out=segmax[:, bc, t * (JT // SEG): t * (JT // SEG) + W // SEG],
                    in_=dp[:, :W].rearrange("p (g e) -> p g e", e=SEG),
                    axis=mybir.AxisListType.X,
                    op=mybir.AluOpType.max,
                )

        # per-row top-8 segments
        for bc in range(BCH):
            val8 = out_pool.tile([128, 8], _F32, tag="val8")
            idx8 = out_pool.tile([128, 8], _U32, tag="idx8")
            nc.vector.max(out=val8[:], in_=segmax[:, bc])
            nc.vector.max_index(out=idx8[:], in_max=val8[:], in_values=segmax[:, bc])
            nc.sync.dma_start(val_out[bc], val8[:])
            nc.sync.dma_start(idx_out[bc], idx8[:])

    nc.compile()
    return nc


def _get_module():
    if "nc" not in _cache:
        _cache["nc"] = _build_module()
    return _cache["nc"]


def kernel(features, predictions, fea_bank, score_bank, trg_idx):
    features = np.asarray(features, dtype=np.float32)
    predictions = np.asarray(predictions, dtype=np.float32)
    fea_bank = np.asarray(fea_bank, dtype=np.float32)
    score_bank = np.asarray(score_bank, dtype=np.float32)
    trg_idx = np.asarray(trg_idx, dtype=np.int32)

    # ---- tiny host prologue (O(B*D)) ----
    sm = predictions - predictions.max(axis=1, keepdims=True)
    np.exp(sm, out=sm)
    sm /= sm.sum(axis=1, keepdims=True)                       # softmax_out [B,C]
    nrm = np.maximum(np.sqrt((features * features).sum(axis=1, keepdims=True)),
                     EPS)
    f_norm = features / nrm                                   # [B,D]

    # bank updates + padding
    fbp = np.zeros((NPAD, D), dtype=np.float32)
    fbp[:N] = fea_bank
    fbp[trg_idx] = f_norm
    sb = score_bank.copy()
    sb[trg_idx] = sm

    # f_norm.T packed as [dp, dc*B + b]
    fnt = np.ascontiguousarray(
        f_norm.T.reshape(4, 128, B).transpose(1, 0, 2).reshape(128, 4 * B))

    nc = _get_module()
    in_maps = [
        {"fb": np.ascontiguousarray(fbp[c * NSHARD:(c + 1) * NSHARD]),
         "fnt": fnt}
        for c in range(NCORES)
    ]
    res = run_bass_kernel_spmd(nc, in_maps, core_ids=list(range(NCORES)))

    # ---- host epilogue: merge candidates, resolve indices, loss ----
    # candidate values / segment ids: [NCORES, B, 8]
    vals = np.stack([r["val_out"].reshape(B, 8) for r in res.results])
    segs = np.stack([r["idx_out"].reshape(B, 8).astype(np.int64)
                     for r in res.results])

    vals = vals.transpose(1, 0, 2).reshape(B, NCORES * 8)     # [B, 64]
    segs = segs.transpose(1, 0, 2).reshape(B, NCORES * 8)
    core_of = np.repeat(np.arange(NCORES, dtype=np.int64), 8)[None, :]
    base = core_of * NSHARD + segs * SEG                      # [B, 64] global row base

    TOP = 8  # resolve a couple extra candidates for tie-order safety
    order = np.argsort(-vals, axis=1, kind="stable")[:, :TOP]
    top_vals = np.take_along_axis(vals, order, axis=1)        # [B, TOP]
    top_base = np.take_along_axis(base, order, axis=1)        # [B, TOP]

    # resolve argmax position within each winning 8-wide segment (fp32 dots)
    rows = top_base[:, :, None] + np.arange(SEG, dtype=np.int64)[None, None, :]
    seg_vecs = fbp[rows.reshape(-1)].reshape(B, TOP, SEG, D)
    dots = np.einsum("rksd,rd->rks", seg_vecs, f_norm, optimize=True)
    pos = dots.argmax(axis=2)                                 # [B, TOP]
    top_idx = top_base + pos                                  # [B, TOP] global rows

    # order exactly like jax.lax.top_k: value desc, index asc on ties
    reorder = np.lexsort((top_idx, -top_vals), axis=1)
    top_idx = np.take_along_axis(top_idx, reorder, axis=1)

    idx_near = top_idx[:, 1:K + 1]                            # drop self slot 0
    score_near = sb[idx_near].astype(np.float64)              # [B,K,C]
    kl = score_near * (np.log(score_near) - sm[:, None, :].astype(np.float64))
    loss = kl.sum(axis=(1, 2)).mean()

    s64 = sm.astype(np.float64)
    neg_pred = (np.square(s64.sum(axis=0)).sum()
                - np.square(s64).sum()) / B

    return np.float32(loss + neg_pred)


# revision 2
# speedup vs baseline: 1.2445x; 1.2445x over previous
"""Trainium2 Bass kernel for nn_AaD_MAPU (retrieval kNN + KL attraction / dispersion loss).

Reference computation:
    softmax_out = softmax(predictions)                      [B,C]
    f_norm      = l2_normalize(features)                    [B,D]
    fb          = fea_bank with rows trg_idx <- f_norm      [N,D]
    sb          = score_bank with rows trg_idx <- softmax   [N,C]
    distance    = f_norm @ fb.T                             [B,N]
    idx         = top_k(distance, K+1); idx_near = idx[:,1:]
    score_near  = sb[idx_near]                              [B,K,C]
    loss        = sum(score_near * (log(score_near) - softmax[:,None,:])) / B
    neg_pred    = mean(rowsum(softmax @ softmax.T - diag))
    out         = loss + neg_pred

Device strategy (8 NeuronCores, bank rows sharded, d-major layout):
  - Pad bank to 100352 rows; core c owns rows [c*12544, (c+1)*12544).
  - The host ships each shard already transposed ([D, 12544] fp32, a pure
    layout change done while sharding) so the contraction dim lands on
    SBUF partitions with contiguous DMA.
  - float32r matmuls (full-rate fp32) vs pre-transposed f_norm.T
    accumulate PSUM fp32 [128b, 512j] tiles.
  - VectorE segmented reduce_max (8-wide segments) straight from PSUM
    -> per-row segment maxima [128, 1568] fp32 per batch chunk.
  - VectorE max8 + find_index8 -> per-row top-8 (segment max, segment id).
Host merges 8 cores x 8 candidates per row, resolves the argmax position
inside each 8-wide winning segment with a handful of fp32 dots, drops the
top-1 (reference drops idx[:,0]), gathers scores and reduces the loss.
"""

from contextlib import ExitStack

import numpy as np

import concourse.bass as bass
import concourse.tile as tile
from concourse import bacc, mybir
from concourse.bass_utils import run_bass_kernel_spmd

# Problem constants (hardcoded per contest rules).
B, D, N, C, K = 512, 512, 100000, 64, 5
EPS = 1e-12
NCORES = 8
NSHARD = 12544            # padded bank rows per core (98 * 128)
NPAD = NSHARD * NCORES    # 100352
SEG = 8                   # segment width for the two-level top-k
NSEG = NSHARD // SEG      # 1568 segments per core per row
BCH = 4                   # batch chunks of 128 rows
JT = 512                  # j-tile width
NJT = 25                  # 24 full tiles + one 256-wide tile

_F32 = mybir.dt.float32
_F32R = mybir.dt.float32r
_U32 = mybir.dt.uint32

_cache = {}


def _build_module():
    nc = bacc.Bacc("TRN2", target_bir_lowering=False, debug=False,
                   num_devices=NCORES)
    # bank shard, transposed on host: [D, NSHARD] fp32 (float32r = same bits)
    fbt_d = nc.dram_tensor("fbt", [D, NSHARD], _F32R, kind="ExternalInput").ap()
    # f_norm.T packed on host as [dp, dc*B + b]
    fnt_d = nc.dram_tensor("fnt", [128, 4 * B], _F32R, kind="ExternalInput").ap()
    val_out = nc.dram_tensor("val_out", [BCH, 128, 8], _F32,
                             kind="ExternalOutput").ap()
    idx_out = nc.dram_tensor("idx_out", [BCH, 128, 8], _U32,
                             kind="ExternalOutput").ap()

    with tile.TileContext(nc) as tc, ExitStack() as ctx:
        const = ctx.enter_context(tc.tile_pool(name="const", bufs=1))
        fbt_pool = ctx.enter_context(tc.tile_pool(name="fbt", bufs=4))
        dp_pool = ctx.enter_context(tc.tile_pool(name="dp", bufs=6, space="PSUM"))
        out_pool = ctx.enter_context(tc.tile_pool(name="outs", bufs=2))

        fnt_sb = const.tile([128, 4, B], _F32R)
        nc.sync.dma_start(fnt_sb[:], fnt_d.rearrange("p (c b) -> p c b", c=4))

        segmax = const.tile([128, BCH, NSEG], _F32)

        for t in range(NJT):
            j0 = t * JT
            W = min(JT, NSHARD - j0)

            # bank tile in [d, j] layout: partition = d % 128, c = d // 128
            fbt = fbt_pool.tile([128, 4, JT], _F32R, tag="fbt")
            nc.sync.dma_start(
                fbt[:, :, :W],
                fbt_d[:, j0:j0 + W].rearrange("(c p) j -> p c j", p=128),
            )

            for bc in range(BCH):
                dp = dp_pool.tile([128, JT], _F32, tag="dp")
                for dc in range(4):
                    nc.tensor.matmul(
                        dp[:, :W],
                        lhsT=fnt_sb[:, dc, bc * 128:(bc + 1) * 128],
                        rhs=fbt[:, dc, :W],
                        start=(dc == 0), stop=(dc == 3),
                    )
                nc.vector.tensor_reduce(
                    out=segmax[:, bc, t * (JT // SEG): t * (JT // SEG) + W // SEG],
                    in_=dp[:, :W].rearrange("p (g e) -> p g e", e=SEG),
                    axis=mybir.AxisListType.X,
                    op=mybir.AluOpType.max,
                )

        # per-row top-8 segments
        for bc in range(BCH):
            val8 = out_pool.tile([128, 8], _F32, tag="val8")
            idx8 = out_pool.tile([128, 8], _U32, tag="idx8")
            nc.vector.max(out=val8[:], in_=segmax[:, bc])
            nc.vector.max_index(out=idx8[:], in_max=val8[:], in_values=segmax[:, bc])
            nc.sync.dma_start(val_out[bc], val8[:])
            nc.sync.dma_start(idx_out[bc], idx8[:])

    nc.compile()
    return nc


def _get_module():
    if "nc" not in _cache:
        _cache["nc"] = _build_module()
    return _cache["nc"]


def kernel(features, predictions, fea_bank, score_bank, trg_idx):
    features = np.asarray(features, dtype=np.float32)
    predictions = np.asarray(predictions, dtype=np.float32)
    fea_bank = np.asarray(fea_bank, dtype=np.float32)
    score_bank = np.asarray(score_bank, dtype=np.float32)
    trg_idx = np.asarray(trg_idx, dtype=np.int32)

    # ---- tiny host prologue (O(B*D)) ----
    sm = predictions - predictions.max(axis=1, keepdims=True)
    np.exp(sm, out=sm)
    sm /= sm.sum(axis=1, keepdims=True)                       # softmax_out [B,C]
    nrm = np.maximum(np.sqrt((features * features).sum(axis=1, keepdims=True)),
                     EPS)
    f_norm = features / nrm                                   # [B,D]

    # bank updates + padding
    fbp = np.zeros((NPAD, D), dtype=np.float32)
    fbp[:N] = fea_bank
    fbp[trg_idx] = f_norm
    sb = score_bank.copy()
    sb[trg_idx] = sm

    # f_norm.T packed as [dp, dc*B + b]
    fnt = np.ascontiguousarray(
        f_norm.T.reshape(4, 128, B).transpose(1, 0, 2).reshape(128, 4 * B))

    nc = _get_module()
    in_maps = [
        {"fbt": np.ascontiguousarray(fbp[c * NSHARD:(c + 1) * NSHARD].T),
         "fnt": fnt}
        for c in range(NCORES)
    ]
    res = run_bass_kernel_spmd(nc, in_maps, core_ids=list(range(NCORES)))

    # ---- host epilogue: merge candidates, resolve indices, loss ----
    vals = np.stack([r["val_out"].reshape(B, 8) for r in res.results])
    segs = np.stack([r["idx_out"].reshape(B, 8).astype(np.int64)
                     for r in res.results])

    vals = vals.transpose(1, 0, 2).reshape(B, NCORES * 8)     # [B, 64]
    segs = segs.transpose(1, 0, 2).reshape(B, NCORES * 8)
    core_of = np.repeat(np.arange(NCORES, dtype=np.int64), 8)[None, :]
    base = core_of * NSHARD + segs * SEG                      # [B, 64] global row base

    TOP = 8  # resolve a couple extra candidates for tie-order safety
    order = np.argsort(-vals, axis=1, kind="stable")[:, :TOP]
    top_vals = np.take_along_axis(vals, order, axis=1)        # [B, TOP]
    top_base = np.take_along_axis(base, order, axis=1)        # [B, TOP]

    # resolve argmax position within each winning 8-wide segment (fp32 dots)
    rows = top_base[:, :, None] + np.arange(SEG, dtype=np.int64)[None, None, :]
    seg_vecs = fbp[rows.reshape(-1)].reshape(B, TOP, SEG, D)
    dots = np.einsum("rksd,rd->rks", seg_vecs, f_norm, optimize=True)
    pos = dots.argmax(axis=2)                                 # [B, TOP]
    top_idx = top_base + pos                                  # [B, TOP] global rows

    # order exactly like jax.lax.top_k: value desc, index asc on ties
    reorder = np.lexsort((top_idx, -top_vals), axis=1)
    top_idx = np.take_along_axis(top_idx, reorder, axis=1)

    idx_near = top_idx[:, 1:K + 1]                            # drop self slot 0
    score_near = sb[idx_near].astype(np.float64)              # [B,K,C]
    kl = score_near * (np.log(score_near) - sm[:, None, :].astype(np.float64))
    loss = kl.sum(axis=(1, 2)).mean()

    s64 = sm.astype(np.float64)
    neg_pred = (np.square(s64.sum(axis=0)).sum()
                - np.square(s64).sum()) / B

    return np.float32(loss + neg_pred)


# revision 4
# speedup vs baseline: 1.4221x; 1.1427x over previous
"""Trainium2 Bass kernel for nn_AaD_MAPU (retrieval kNN + KL attraction / dispersion loss).

Reference computation:
    softmax_out = softmax(predictions)                      [B,C]
    f_norm      = l2_normalize(features)                    [B,D]
    fb          = fea_bank with rows trg_idx <- f_norm      [N,D]
    sb          = score_bank with rows trg_idx <- softmax   [N,C]
    distance    = f_norm @ fb.T                             [B,N]
    idx         = top_k(distance, K+1); idx_near = idx[:,1:]
    score_near  = sb[idx_near]                              [B,K,C]
    loss        = sum(score_near * (log(score_near) - softmax[:,None,:])) / B
    neg_pred    = mean(rowsum(softmax @ softmax.T - diag))
    out         = loss + neg_pred

Device strategy (8 NeuronCores, bank rows sharded, d-major layout):
  - Pad bank to 100352 rows; core c owns rows [c*12544, (c+1)*12544).
  - The host ships each shard already transposed ([D, 12544] fp32, a pure
    layout change done while sharding) so the contraction dim lands on
    SBUF partitions with contiguous DMA.
  - float32r matmuls (full-rate fp32) vs pre-transposed f_norm.T
    accumulate PSUM fp32 [128b, 512j] tiles; a handful of zero warm-up
    matmuls during the first DMA bring the PE out of its cold p-state.
  - VectorE segmented reduce_max (8-wide segments) straight from PSUM
    -> per-row segment maxima [128, 1568] fp32 per batch chunk.
  - VectorE max8 + find_index8 per half (the first half fires while the
    matmul loop is still running) -> per-row top-8 (segment max, id) per
    half per 128-row chunk.
Host merges 8 cores x 16 candidates per row, resolves the argmax position
inside each 8-wide winning segment with a handful of fp32 dots, drops the
top-1 (reference drops idx[:,0]), gathers scores and reduces the loss.
"""

from contextlib import ExitStack

import numpy as np

import concourse.bass as bass
import concourse.tile as tile
from concourse import bacc, mybir
from concourse.bass_utils import run_bass_kernel_spmd

# Problem constants (hardcoded per contest rules).
B, D, N, C, K = 512, 512, 100000, 64, 5
EPS = 1e-12
NCORES = 8
NSHARD = 12544            # padded bank rows per core (98 * 128)
NPAD = NSHARD * NCORES    # 100352
SEG = 8                   # segment width for the two-level top-k
NSEG = NSHARD // SEG      # 1568 segments per core per row
BCH = 4                   # batch chunks of 128 rows
JT = 512                  # j-tile width
NJT = 25                  # 24 full tiles + one 256-wide tile
HALF0 = 768               # segments in first half (12 j-tiles)
HALF_TILE = 12            # first half complete after this many tiles
N_WARMUP = 16             # zero matmuls to warm the PE during the first DMA

_F32 = mybir.dt.float32
_F32R = mybir.dt.float32r
_U32 = mybir.dt.uint32

_cache = {}


def _build_module():
    nc = bacc.Bacc("TRN2", target_bir_lowering=False, debug=False,
                   num_devices=NCORES)
    # bank shard, transposed on host: [D, NSHARD] fp32 (float32r = same bits)
    fbt_d = nc.dram_tensor("fbt", [D, NSHARD], _F32R, kind="ExternalInput").ap()
    # f_norm.T packed on host as [dp, dc*B + b]
    fnt_d = nc.dram_tensor("fnt", [128, 4 * B], _F32R, kind="ExternalInput").ap()
    # top-8 per (128-row chunk, half): value and segment id, half-major inner
    val_out = nc.dram_tensor("val_out", [128, BCH, 2, 8], _F32,
                             kind="ExternalOutput").ap()
    idx_out = nc.dram_tensor("idx_out", [128, BCH, 2, 8], _U32,
                             kind="ExternalOutput").ap()

    with tile.TileContext(nc) as tc, ExitStack() as ctx:
        const = ctx.enter_context(tc.tile_pool(name="const", bufs=1))
        fbt_pool = ctx.enter_context(tc.tile_pool(name="fbt", bufs=4))
        dp_pool = ctx.enter_context(tc.tile_pool(name="dp", bufs=3, space="PSUM"))
        wu_pool = ctx.enter_context(tc.tile_pool(name="wu", bufs=1, space="PSUM"))
        out_pool = ctx.enter_context(tc.tile_pool(name="outs", bufs=1))

        # PE warm-up: harmless zero matmuls that run while the first DMAs land
        wu_sb = const.tile([128, JT], _F32)
        nc.gpsimd.memset(wu_sb[:], 0.0)
        wu_ps = wu_pool.tile([128, JT], _F32)
        wu_r = wu_sb[:].bitcast(_F32R)
        for _ in range(N_WARMUP):
            nc.tensor.matmul(wu_ps[:], lhsT=wu_r[:, :128], rhs=wu_r,
                             start=True, stop=True)

        fnt_sb = [const.tile([128, B], _F32R, name=f"fnt{dc}") for dc in range(4)]
        for dc in range(4):
            nc.sync.dma_start(fnt_sb[dc][:], fnt_d[:, dc * B:(dc + 1) * B])

        segmax = const.tile([128, BCH, NSEG], _F32)
        vcat = out_pool.tile([128, BCH, 2, 8], _F32)
        icat = out_pool.tile([128, BCH, 2, 8], _U32)

        def top8(bc, half):
            lo, hi = (0, HALF0) if half == 0 else (HALF0, NSEG)
            sl = segmax[:, bc, lo:hi]
            nc.vector.max(out=vcat[:, bc, half], in_=sl)
            nc.vector.max_index(out=icat[:, bc, half], in_max=vcat[:, bc, half],
                                in_values=sl)

        for t in range(NJT):
            j0 = t * JT
            W = min(JT, NSHARD - j0)

            # bank tile in [d, j] layout: partition = d % 128, c = d // 128
            fbt = fbt_pool.tile([128, 4, JT], _F32R, tag="fbt")
            nc.sync.dma_start(
                fbt[:, :, :W],
                fbt_d[:, j0:j0 + W].rearrange("(c p) j -> p c j", p=128),
            )

            for bcp in range(2):          # pairs of 128-row batch chunks
                dp = dp_pool.tile([128, 2, JT], _F32, tag="dp")
                for i in range(2):
                    bc = bcp * 2 + i
                    for dc in range(4):
                        nc.tensor.matmul(
                            dp[:, i, :W],
                            lhsT=fnt_sb[dc][:, bc * 128:(bc + 1) * 128],
                            rhs=fbt[:, dc, :W],
                            start=(dc == 0), stop=(dc == 3),
                        )
                nc.vector.tensor_reduce(
                    out=segmax[:, bcp * 2:bcp * 2 + 2,
                               t * (JT // SEG): t * (JT // SEG) + W // SEG],
                    in_=dp[:, :, :W].rearrange("p i (g e) -> p i g e", e=SEG),
                    axis=mybir.AxisListType.X,
                    op=mybir.AluOpType.max,
                )

            if t == HALF_TILE - 1:        # first 768 segments are final
                for bc in range(BCH):
                    top8(bc, 0)

        for bc in range(BCH):
            top8(bc, 1)
        nc.sync.dma_start(val_out, vcat[:])
        nc.sync.dma_start(idx_out, icat[:])

    nc.compile()
    return nc


def _get_module():
    if "nc" not in _cache:
        _cache["nc"] = _build_module()
    return _cache["nc"]


def kernel(features, predictions, fea_bank, score_bank, trg_idx):
    features = np.asarray(features, dtype=np.float32)
    predictions = np.asarray(predictions, dtype=np.float32)
    fea_bank = np.asarray(fea_bank, dtype=np.float32)
    score_bank = np.asarray(score_bank, dtype=np.float32)
    trg_idx = np.asarray(trg_idx, dtype=np.int32)

    # ---- tiny host prologue (O(B*D)) ----
    sm = predictions - predictions.max(axis=1, keepdims=True)
    np.exp(sm, out=sm)
    sm /= sm.sum(axis=1, keepdims=True)                       # softmax_out [B,C]
    nrm = np.maximum(np.sqrt((features * features).sum(axis=1, keepdims=True)),
                     EPS)
    f_norm = features / nrm                                   # [B,D]

    # bank updates + padding
    fbp = np.zeros((NPAD, D), dtype=np.float32)
    fbp[:N] = fea_bank
    fbp[trg_idx] = f_norm
    sb = score_bank.copy()
    sb[trg_idx] = sm

    # f_norm.T packed as [dp, dc*B + b]
    fnt = np.ascontiguousarray(
        f_norm.T.reshape(4, 128, B).transpose(1, 0, 2).reshape(128, 4 * B))

    nc = _get_module()
    in_maps = [
        {"fbt": np.ascontiguousarray(fbp[c * NSHARD:(c + 1) * NSHARD].T),
         "fnt": fnt}
        for c in range(NCORES)
    ]
    res = run_bass_kernel_spmd(nc, in_maps, core_ids=list(range(NCORES)))

    # ---- host epilogue: merge candidates, resolve indices, loss ----
    # outputs are [128, BCH, 2, 8]; row b = bc*128 + p
    CAND = 16
    vals = np.empty((B, NCORES * CAND), np.float32)
    base = np.empty((B, NCORES * CAND), np.int64)
    half_off = np.array([0, HALF0], np.int64)[None, :, None]
    for c, r in enumerate(res.results):
        v = r["val_out"].transpose(1, 0, 2, 3).reshape(B, CAND)
        s = r["idx_out"].astype(np.int64).transpose(1, 0, 2, 3)
        s = (s + half_off[None]).reshape(B, CAND)
        vals[:, c * CAND:(c + 1) * CAND] = v
        base[:, c * CAND:(c + 1) * CAND] = c * NSHARD + s * SEG

    TOP = 8  # resolve a couple extra candidates for tie-order safety
    order = np.argsort(-vals, axis=1, kind="stable")[:, :TOP]
    top_vals = np.take_along_axis(vals, order, axis=1)        # [B, TOP]
    top_base = np.take_along_axis(base, order, axis=1)        # [B, TOP]

    # resolve argmax position within each winning 8-wide segment (fp32 dots)
    rows = top_base[:, :, None] + np.arange(SEG, dtype=np.int64)[None, None, :]
    seg_vecs = fbp[rows.reshape(-1)].reshape(B, TOP, SEG, D)
    dots = np.einsum("rksd,rd->rks", seg_vecs, f_norm, optimize=True)
    pos = dots.argmax(axis=2)                                 # [B, TOP]
    top_idx = top_base + pos                                  # [B, TOP] global rows

    # order exactly like jax.lax.top_k: value desc, index asc on ties
    reorder = np.lexsort((top_idx, -top_vals), axis=1)
    top_idx = np.take_along_axis(top_idx, reorder, axis=1)

    idx_near = top_idx[:, 1:K + 1]                            # drop self slot 0
    score_near = sb[idx_near].astype(np.float64)              # [B,K,C]
    kl = score_near * (np.log(score_near) - sm[:, None, :].astype(np.float64))
    loss = kl.sum(axis=(1, 2)).mean()

    s64 = sm.astype(np.float64)
    neg_pred = (np.square(s64.sum(axis=0)).sum()
                - np.square(s64).sum()) / B

    return np.float32(loss + neg_pred)


# revision 7
# speedup vs baseline: 1.5118x; 1.0630x over previous
"""Trainium2 Bass kernel for nn_AaD_MAPU (retrieval kNN + KL attraction / dispersion loss).

Reference computation:
    softmax_out = softmax(predictions)                      [B,C]
    f_norm      = l2_normalize(features)                    [B,D]
    fb          = fea_bank with rows trg_idx <- f_norm      [N,D]
    sb          = score_bank with rows trg_idx <- softmax   [N,C]
    distance    = f_norm @ fb.T                             [B,N]
    idx         = top_k(distance, K+1); idx_near = idx[:,1:]
    score_near  = sb[idx_near]                              [B,K,C]
    loss        = sum(score_near * (log(score_near) - softmax[:,None,:])) / B
    neg_pred    = mean(rowsum(softmax @ softmax.T - diag))
    out         = loss + neg_pred

Device strategy (8 NeuronCores, bank rows sharded, d-major layout):
  - Pad bank to 100352 rows; core c owns rows [c*12544, (c+1)*12544).
  - The host ships each shard already transposed ([D, 12544] fp32, a pure
    layout change done while sharding) so the contraction dim lands on
    SBUF partitions with contiguous DMA.
  - float32r matmuls (full-rate fp32) vs pre-transposed f_norm.T
    accumulate PSUM fp32 [128b, 512j] tiles; a handful of zero warm-up
    matmuls during the first DMA bring the PE out of its cold p-state.
  - VectorE segmented reduce_max (8-wide segments) straight from PSUM
    -> per-row segment maxima [128, 1568] fp32 per batch chunk.
  - VectorE max8 + find_index8 per half (the first half fires while the
    matmul loop is still running) -> per-row top-8 (segment max, id) per
    half per 128-row chunk.
Host merges 8 cores x 16 candidates per row, resolves the argmax position
inside each 8-wide winning segment with a handful of fp32 dots, drops the
top-1 (reference drops idx[:,0]), gathers scores and reduces the loss.
"""

from contextlib import ExitStack

import numpy as np

import concourse.bass as bass
import concourse.tile as tile
from concourse import bacc, mybir
from concourse.bass_utils import run_bass_kernel_spmd

# Problem constants (hardcoded per contest rules).
B, D, N, C, K = 512, 512, 100000, 64, 5
EPS = 1e-12
NCORES = 8
NSHARD = 12544            # padded bank rows per core (98 * 128)
NPAD = NSHARD * NCORES    # 100352
SEG = 8                   # segment width for the two-level top-k
NSEG = NSHARD // SEG      # 1568 segments per core per row
BCH = 4                   # batch chunks of 128 rows
JT = 512                  # j-tile width
NJT = 25                  # 24 full tiles + one 256-wide tile
# segmax is split in three parts; each part's top-8 extraction starts as
# soon as its segments are final, hiding most of the work under the matmuls
SPLIT_TILES = (10, 20, NJT)           # part boundaries, in j-tiles
SPLIT_SEGS = (0, 640, 1280, NSEG)     # corresponding segment offsets
PARTS = 3
N_WARMUP = 16             # zero matmuls to warm the PE during the first DMA

_F32 = mybir.dt.float32
_F32R = mybir.dt.float32r
_U32 = mybir.dt.uint32

_cache = {}


def _build_module():
    nc = bacc.Bacc("TRN2", target_bir_lowering=False, debug=False,
                   num_devices=NCORES)
    # bank shard, transposed on host: [D, NSHARD] fp32 (float32r = same bits)
    fbt_d = nc.dram_tensor("fbt", [D, NSHARD], _F32R, kind="ExternalInput").ap()
    # f_norm.T packed on host as [dp, dc*B + b]
    fnt_d = nc.dram_tensor("fnt", [128, 4 * B], _F32R, kind="ExternalInput").ap()
    # top-8 per (128-row chunk, part): value and segment id
    val_out = nc.dram_tensor("val_out", [128, BCH, PARTS, 8], _F32,
                             kind="ExternalOutput").ap()
    idx_out = nc.dram_tensor("idx_out", [128, BCH, PARTS, 8], _U32,
                             kind="ExternalOutput").ap()

    with tile.TileContext(nc) as tc, ExitStack() as ctx:
        const = ctx.enter_context(tc.tile_pool(name="const", bufs=1))
        fbt_pool = ctx.enter_context(tc.tile_pool(name="fbt", bufs=4))
        dp_pool = ctx.enter_context(tc.tile_pool(name="dp", bufs=4, space="PSUM"))
        out_pool = ctx.enter_context(tc.tile_pool(name="outs", bufs=1))

        # PE warm-up: harmless zero matmuls that run while the first DMAs land
        wu_sb = const.tile([128, JT], _F32)
        nc.gpsimd.memset(wu_sb[:], 0.0)
        wu_ps = dp_pool.tile([128, 2, JT], _F32, tag="dp")
        wu_r = wu_sb[:].bitcast(_F32R)
        for _ in range(N_WARMUP):
            nc.tensor.matmul(wu_ps[:, 0], lhsT=wu_r[:, :128], rhs=wu_r,
                             start=True, stop=True)

        fnt_sb = [const.tile([128, B], _F32R, name=f"fnt{dc}") for dc in range(4)]
        for dc in range(4):
            nc.sync.dma_start(fnt_sb[dc][:], fnt_d[:, dc * B:(dc + 1) * B])

        segmax = const.tile([128, BCH, NSEG], _F32)
        vcat = out_pool.tile([128, BCH, PARTS, 8], _F32)
        icat = out_pool.tile([128, BCH, PARTS, 8], _U32)

        def top8(bc, part):
            lo, hi = SPLIT_SEGS[part], SPLIT_SEGS[part + 1]
            sl = segmax[:, bc, lo:hi]
            nc.vector.max(out=vcat[:, bc, part], in_=sl)
            nc.vector.max_index(out=icat[:, bc, part], in_max=vcat[:, bc, part],
                                in_values=sl)

        for t in range(NJT):
            j0 = t * JT
            W = min(JT, NSHARD - j0)

            # bank tile in [d, j] layout: partition = d % 128, c = d // 128
            fbt = fbt_pool.tile([128, 4, JT], _F32R, tag="fbt")
            nc.sync.dma_start(
                fbt[:, :, :W],
                fbt_d[:, j0:j0 + W].rearrange("(c p) j -> p c j", p=128),
            )

            for bcp in range(2):          # pairs of 128-row batch chunks
                dp = dp_pool.tile([128, 2, JT], _F32, tag="dp")
                for i in range(2):
                    bc = bcp * 2 + i
                    for dc in range(4):
                        nc.tensor.matmul(
                            dp[:, i, :W],
                            lhsT=fnt_sb[dc][:, bc * 128:(bc + 1) * 128],
                            rhs=fbt[:, dc, :W],
                            start=(dc == 0), stop=(dc == 3),
                        )
                nc.vector.tensor_reduce(
                    out=segmax[:, bcp * 2:bcp * 2 + 2,
                               t * (JT // SEG): t * (JT // SEG) + W // SEG],
                    in_=dp[:, :, :W].rearrange("p i (g e) -> p i g e", e=SEG),
                    axis=mybir.AxisListType.X,
                    op=mybir.AluOpType.max,
                )

            # spread finished parts' top-8 extraction across later tiles
            for part in range(PARTS - 1):
                if SPLIT_TILES[part] <= t < SPLIT_TILES[part] + BCH:
                    top8(t - SPLIT_TILES[part], part)

        for bc in range(BCH):
            top8(bc, PARTS - 1)
        nc.sync.dma_start(val_out, vcat[:])
        nc.sync.dma_start(idx_out, icat[:])

    nc.compile()
    return nc


def _get_module():
    if "nc" not in _cache:
        _cache["nc"] = _build_module()
    return _cache["nc"]


def kernel(features, predictions, fea_bank, score_bank, trg_idx):
    features = np.asarray(features, dtype=np.float32)
    predictions = np.asarray(predictions, dtype=np.float32)
    fea_bank = np.asarray(fea_bank, dtype=np.float32)
    score_bank = np.asarray(score_bank, dtype=np.float32)
    trg_idx = np.asarray(trg_idx, dtype=np.int32)

    # ---- tiny host prologue (O(B*D)) ----
    sm = predictions - predictions.max(axis=1, keepdims=True)
    np.exp(sm, out=sm)
    sm /= sm.sum(axis=1, keepdims=True)                       # softmax_out [B,C]
    nrm = np.maximum(np.sqrt((features * features).sum(axis=1, keepdims=True)),
                     EPS)
    f_norm = features / nrm                                   # [B,D]

    # bank updates + padding
    fbp = np.zeros((NPAD, D), dtype=np.float32)
    fbp[:N] = fea_bank
    fbp[trg_idx] = f_norm
    sb = score_bank.copy()
    sb[trg_idx] = sm

    # f_norm.T packed as [dp, dc*B + b]
    fnt = np.ascontiguousarray(
        f_norm.T.reshape(4, 128, B).transpose(1, 0, 2).reshape(128, 4 * B))

    nc = _get_module()
    in_maps = [
        {"fbt": np.ascontiguousarray(fbp[c * NSHARD:(c + 1) * NSHARD].T),
         "fnt": fnt}
        for c in range(NCORES)
    ]
    res = run_bass_kernel_spmd(nc, in_maps, core_ids=list(range(NCORES)))

    # ---- host epilogue: merge candidates, resolve indices, loss ----
    # outputs are [128, BCH, PARTS, 8]; row b = bc*128 + p
    CAND = PARTS * 8
    vals = np.empty((B, NCORES * CAND), np.float32)
    base = np.empty((B, NCORES * CAND), np.int64)
    part_off = np.asarray(SPLIT_SEGS[:PARTS], np.int64)[None, :, None]
    for c, r in enumerate(res.results):
        v = r["val_out"].transpose(1, 0, 2, 3).reshape(B, CAND)
        s = r["idx_out"].astype(np.int64).transpose(1, 0, 2, 3)
        s = (s + part_off[None]).reshape(B, CAND)
        vals[:, c * CAND:(c + 1) * CAND] = v
        base[:, c * CAND:(c + 1) * CAND] = c * NSHARD + s * SEG

    TOP = 8  # resolve a couple extra candidates for tie-order safety
    order = np.argsort(-vals, axis=1, kind="stable")[:, :TOP]
    top_vals = np.take_along_axis(vals, order, axis=1)        # [B, TOP]
    top_base = np.take_along_axis(base, order, axis=1)        # [B, TOP]

    # resolve argmax position within each winning 8-wide segment (fp32 dots)
    rows = top_base[:, :, None] + np.arange(SEG, dtype=np.int64)[None, None, :]
    seg_vecs = fbp[rows.reshape(-1)].reshape(B, TOP, SEG, D)
    dots = np.einsum("rksd,rd->rks", seg_vecs, f_norm, optimize=True)
    pos = dots.argmax(axis=2)                                 # [B, TOP]
    top_idx = top_base + pos                                  # [B, TOP] global rows

    # order exactly like jax.lax.top_k: value desc, index asc on ties
    reorder = np.lexsort((top_idx, -top_vals), axis=1)
    top_idx = np.take_along_axis(top_idx, reorder, axis=1)

    idx_near = top_idx[:, 1:K + 1]                            # drop self slot 0
    score_near = sb[idx_near].astype(np.float64)              # [B,K,C]
    kl = score_near * (np.log(score_near) - sm[:, None, :].astype(np.float64))
    loss = kl.sum(axis=(1, 2)).mean()

    s64 = sm.astype(np.float64)
    neg_pred = (np.square(s64.sum(axis=0)).sum()
                - np.square(s64).sum()) / B

    return np.float32(loss + neg_pred)
